# revision 21
# baseline (speedup 1.0000x reference)
"""GroupedQueryAttention (B=1, T=2048, D=4096, 32 q-heads / 8 kv-heads, hd=128)
on 8 trn2 NeuronCores.

Sharding: kv-head parallel for QKV+attention (core c owns kv head c and its
4 query heads), then sequence-parallel for the output projection. Two
pipelined AllToAlls redistribute y from head-sharded to T-sharded (8x less
wire than an AllGather): A2A#1 covers T columns [0,1024) and fires mid-
attention; each half's y-load + softmax normalization runs right after its
A2A, so only the wo matmuls trail A2A#2. Core d owns T columns
[128d,128d+128) and [1024+128d,1024+128d+128).

All big matmuls run in bf16 (fp32 is 4 cycles/row on the PE, bf16 is 1).
Attention chunks are interleaved with the projection chunks in program
order (chunk qc of attention only needs phase-1 chunks 0..qc), causal
score/exp/yacc work is column-sliced to skip fully-masked regions, and y
ships UNNORMALIZED through the A2A with its softmax denominators riding as
4 extra bf16 rows per block (partition_all_reduce on the idle GpSimd, off
the critical path). Softmax runs without max-subtraction: fp32 exp can't
overflow at these score magnitudes.

Pipeline/latency structure (tuned against the TimelineSim cost model):
- rope avoids the ACT half-swap copies: DVE reads the psum halves at
  partition offsets and multiplies by sin directly (ACT was the local
  bottleneck engine during p1/attention overlap).
- causal diagonal tiles are masked per-tile right after their exp (deps
  are byte-range precise), so each diagonal y-matmul waits only on its
  own tile instead of a batched all-tiles mask.
- the softmax-denominator tree for the heads that gate an AllToAll writes
  a fresh tile (pure read of e) so it overlaps the y matmuls instead of
  trailing them; denominators are batched into one [4,512] tile per chunk
  with a single DMA (SP DMA-issue serialization was on the a2a path).
- phase 4 emits all half-0 (ready right after A2A#1) matmuls of a wo
  chunk before the half-1 ones, and group 0 runs half-0 ONLY with its
  half-1 pass deferred to the end against a reloaded wo stream (+8MB HBM,
  DMA has slack): the in-order PE queue gets a ~14us runway of ready work
  to chew while A2A#2 + normalize complete.
- wo streams through the same SBUF rings as the phase-1 x tiles (their
  lifetimes are disjoint), freeing SBUF for the above.
"""
import sys

sys.path.insert(0, "/opt/trn_rl_repo")

import numpy as np
import ml_dtypes

import concourse.bacc as bacc
import concourse.tile as tile
from concourse import bass_isa, mybir
from concourse.bass_utils import run_bass_kernel_spmd
from concourse.masks import make_identity

N_CORES = 8
T = 2048
DIM = 4096
HD = 128
NH = 32
NKV = 8
NREP = NH // NKV  # 4 query heads per core
NCHUNK = T // 512  # 4 chunks of 512 along T
NKT = DIM // 128  # 32 contraction tiles for the projections
F32 = mybir.dt.float32
BF16 = mybir.dt.bfloat16
NPBF16 = ml_dtypes.bfloat16
SCALE = 1.0 / float(np.sqrt(HD))

import os

MAXPHASE = int(os.environ.get("GQA_MAXPHASE", "4"))
NOL = bool(int(os.environ.get("GQA_NOL", "0")))  # timing diagnostic only
# TimelineSim can't model collectives; swap the A2A for a same-size local
# DRAM->DRAM DMA so the single-core timeline is still representative.
LOCALA2A = bool(int(os.environ.get("GQA_LOCALA2A", "0")))
# Benchmarking aid: build the NEFF with the whole kernel body repeated k
# times; wall-time differencing between k builds isolates HW exec time.
ITERS = int(os.environ.get("GQA_ITERS", "1"))

_cached = {}


def _build_kernel():
    if "nc" in _cached:
        return _cached["nc"]

    nc = bacc.Bacc("TRN2", target_bir_lowering=False)

    xT = nc.dram_tensor("xT", [DIM, T], BF16, kind="ExternalInput")
    cos2 = nc.dram_tensor("cos2", [128, T], F32, kind="ExternalInput")
    sin2 = nc.dram_tensor("sin2", [128, T], F32, kind="ExternalInput")
    masks = nc.dram_tensor("masks", [128, 4 * 512], BF16, kind="ExternalInput")
    wqT = nc.dram_tensor("wqT", [DIM, NREP * HD], BF16, kind="ExternalInput")
    wkvT = nc.dram_tensor("wkvT", [DIM, 2 * HD], BF16, kind="ExternalInput")
    woT = nc.dram_tensor("woT", [DIM, DIM], BF16, kind="ExternalInput")
    out = nc.dram_tensor("out", [256, DIM], F32, kind="ExternalOutput")

    # AllToAll buffers, one pair per T-half. 8 row-blocks of 516: block d on
    # core c holds core c's 512 head-dims of UNNORMALIZED y for global column
    # block 128*(8*half+d)..+128, plus 4 rows of softmax denominators (one per
    # head, bf16). Normalization happens after the A2A, folded into phase 4.
    BLK = 512 + NREP  # 516
    y_in = [
        nc.dram_tensor(f"y_in{i}", [N_CORES * BLK, 128], BF16, kind="Internal")
        for i in range(2)
    ]
    y_out = [
        nc.dram_tensor(f"y_out{i}", [N_CORES * BLK, 128], BF16, kind="Internal")
        for i in range(2)
    ]

    with tile.TileContext(nc) as tc:
        with (
            tc.tile_pool(name="consts", bufs=1) as consts,
            tc.tile_pool(name="weights", bufs=1) as weights,
            tc.tile_pool(name="acts", bufs=1) as acts,
            tc.tile_pool(name="stream", bufs=1) as stream,
            tc.tile_pool(name="work", bufs=2) as work,
            tc.tile_pool(name="expp", bufs=2) as expp,
            tc.tile_pool(name="outp", bufs=2) as outp,
            tc.tile_pool(name="psum", bufs=8, space="PSUM") as psum,
        ):
            for _it in range(ITERS):
                def load_x(qc, first=None):
                    x_ck = []
                    for i in range(4):
                        if first is not None and i == 0:
                            x_ck.append(first)
                            continue
                        t = stream.tile(
                            [128, 8, 512], BF16, tag=f"x{i % 2}", name=f"x{qc}_{i}"
                        )
                        nc.sync.dma_start(
                            out=t,
                            in_=xT[
                                1024 * i:1024 * (i + 1),
                                512 * qc:512 * (qc + 1),
                            ].rearrange("(n p) m -> p n m", p=128),
                        )
                        x_ck.append(t)
                    return x_ck

                # ---------- resident weights, split + multi-queue so the
                # first chunk's matmuls (k/v first) start as early as possible
                wkv_sb = weights.tile([128, NKT, 2 * HD], BF16, tag="wkv")
                nc.sync.dma_start(
                    out=wkv_sb[:, 0:8, :],
                    in_=wkvT[0:1024, :].rearrange("(n p) m -> p n m", p=128),
                )
                x0 = stream.tile([128, 8, 512], BF16, tag="x0", name="x0_0")
                nc.scalar.dma_start(
                    out=x0[:, 0:2, :],
                    in_=xT[0:256, 0:512].rearrange("(n p) m -> p n m", p=128),
                )
                nc.scalar.dma_start(
                    out=x0[:, 2:4, :],
                    in_=xT[256:512, 0:512].rearrange("(n p) m -> p n m", p=128),
                )
                wq_sb = [
                    weights.tile([128, 8, NREP * HD], BF16, tag=f"wq{i}",
                                 name=f"wq{i}")
                    for i in range(4)
                ]
                nc.sync.dma_start(
                    out=wq_sb[0][:, 0:4, :],
                    in_=wqT[0:512, :].rearrange("(n p) m -> p n m", p=128),
                )
                nc.scalar.dma_start(
                    out=x0[:, 4:8, :],
                    in_=xT[512:1024, 0:512].rearrange("(n p) m -> p n m", p=128),
                )
                nc.sync.dma_start(
                    out=wkv_sb[:, 8:32, :],
                    in_=wkvT[1024:4096, :].rearrange("(n p) m -> p n m", p=128),
                )
                nc.sync.dma_start(
                    out=wq_sb[0][:, 4:8, :],
                    in_=wqT[512:1024, :].rearrange("(n p) m -> p n m", p=128),
                )
                x_pre = load_x(0, first=x0)
                for i in range(1, 4):
                    nc.sync.dma_start(
                        out=wq_sb[i],
                        in_=wqT.rearrange("(n p) m -> p n m", p=128)[
                            :, 8 * i:8 * (i + 1), :
                        ],
                    )

                # constants: not needed until the first rope, ~40us in
                cos_sb = consts.tile([128, T], F32, tag="cos")
                nc.sync.dma_start(out=cos_sb, in_=cos2[:, :])
                sin_sb = consts.tile([128, T], F32, tag="sin")
                nc.sync.dma_start(out=sin_sb, in_=sin2[:, :])
                mask_sb = consts.tile([128, 4, 512], BF16, tag="mask")
                nc.sync.dma_start(
                    out=mask_sb, in_=masks.rearrange("p (d m) -> p d m", d=4)
                )
                ident = consts.tile([128, 128], BF16, tag="ident")
                make_identity(nc, ident)

                # activations that live through the attention phase —
                # PER-CHUNK tiles so attention chunk qc only depends on the
                # phase-1 chunks it actually reads (Tile deps are per-tile)
                qT_t = [
                    acts.tile([128, NREP, 512], BF16, tag=f"qt{i}", name=f"qt{i}")
                    for i in range(NCHUNK)
                ]
                kT_t = [
                    acts.tile([128, 512], BF16, tag=f"kt{i}", name=f"kt{i}")
                    for i in range(NCHUNK)
                ]
                vkd_t = [
                    acts.tile([128, 4, HD], BF16, tag=f"vkd{i}", name=f"vkd{i}")
                    for i in range(NCHUNK)
                ]

                # ---------- phase 1: QKV projections + rope ----------
                # x is chunk-resident (two 2 MB DMAs per 512-col chunk)
                def p1_chunk(qc):
                    x_ck = x_pre if qc == 0 else load_x(qc)
                    cs = slice(512 * qc, 512 * (qc + 1))
                    q_ps = [
                        psum.tile([128, 512], F32, tag="bank", name=f"qps{qc}_{h}")
                        for h in range(NREP)
                    ]
                    k_ps = psum.tile([128, 512], F32, tag="bank")
                    v_ps = psum.tile([128, 512], F32, tag="bank")
                    for kt in range(NKT):
                        xt = x_ck[kt // 8][:, kt % 8, :]
                        st = kt == 0
                        sp = kt == NKT - 1
                        nc.tensor.matmul(
                            k_ps, lhsT=wkv_sb[:, kt, 0:HD], rhs=xt, start=st, stop=sp
                        )
                        nc.tensor.matmul(
                            v_ps, lhsT=wkv_sb[:, kt, HD:2 * HD], rhs=xt, start=st, stop=sp
                        )
                        for h in range(NREP):
                            nc.tensor.matmul(
                                q_ps[h],
                                lhsT=wq_sb[kt // 8][:, kt % 8, 128 * h:128 * (h + 1)],
                                rhs=xt,
                                start=st,
                                stop=sp,
                            )

                    # v computed in [hd, T] layout; transpose 128x128 blocks
                    v_sb = work.tile([128, 512], BF16, tag="vsb")
                    nc.scalar.copy(v_sb, v_ps)
                    for s in range(4):
                        vt_ps = psum.tile(
                            [128, 128], BF16, tag="bank", name=f"vt{qc}_{s}"
                        )
                        with nc.allow_low_precision(reason="pure transpose, no accumulation"):
                            nc.tensor.transpose(
                                vt_ps, v_sb[:, 128 * s:128 * (s + 1)], ident
                            )
                        nc.scalar.copy(vkd_t[qc][:, s, :], vt_ps)

                    # rope for the 4 q heads and k (fp32 math, bf16 store).
                    # The rotate-half swap is folded into the sin multiply:
                    # DVE reads the psum halves at +/-64-partition offsets,
                    # keeping the swap off the (saturated) ACT engine.
                    for h in range(NREP + 1):
                        p = q_ps[h] if h < NREP else k_ps
                        dst = qT_t[qc][:, h, :] if h < NREP else kT_t[qc][:, :]
                        # the rotate-half swap must be on ACT: the walrus
                        # verifier requires TensorTensor operands to share
                        # their start partition, but ACT copies may cross
                        sw = work.tile([128, 512], F32, tag="sw")
                        nc.scalar.copy(sw[0:64, :], p[64:128, :])
                        nc.scalar.copy(sw[64:128, :], p[0:64, :])
                        nc.vector.tensor_mul(sw, sw, sin_sb[:, cs])
                        d1 = work.tile([128, 512], F32, tag="lred")
                        nc.vector.tensor_mul(d1, p, cos_sb[:, cs])
                        nc.vector.tensor_add(dst, d1, sw)

                # ---------- phase 2 + 3: attention, A2A#1 after first T-half ----------
                # Chunks 0/1 have tiny PE work but long exp->mask chains, so
                # their emission is split: score matmuls + exp/mask (part A)
                # fire before the next projection chunk, and the y matmuls
                # (part B) after it — the chains complete in the shadow of
                # ~40us of projection matmuls instead of head-of-line
                # blocking the in-order PE queue. Their e tiles are packed
                # (4 resp. 2 heads per [128,16,512] ring tile) so part A
                # never waits on part B through the tile ring.
                e_reg = {}
                ft_reg = {}
                l4_reg = {}

                def att_etile(qc, h):
                    if qc == 0:
                        if 0 not in e_reg:
                            e_reg[0] = expp.tile(
                                [128, 16, 512], BF16, tag="efull", name="ef0"
                            )
                        return e_reg[0], 4 * h
                    if qc == 1:
                        if 1 not in e_reg:
                            e_reg[1] = [
                                expp.tile([128, 16, 512], BF16, tag="efull",
                                          name="ef1a"),
                                expp.tile([128, 16, 512], BF16, tag="efull",
                                          name="ef1b"),
                            ]
                        return e_reg[1][h // 2], 8 * (h % 2)
                    key = (qc, h)
                    if key not in e_reg:
                        e_reg[key] = expp.tile(
                            [128, 16, 512], BF16, tag="efull", name=f"ef{qc}_{h}"
                        )
                    return e_reg[key], 0

                def att_A(qc, heads):
                    nkt = 4 * qc + 4  # causal: k tiles 0 .. 4*qc+3
                    for h in heads:
                        et, b = att_etile(qc, h)
                        # heads whose l gates (or queues just before) an
                        # a2a accumulate their softmax denominator
                        # incrementally (4 ftree lanes) while tiles stream,
                        # so only a 2-level tree trails the last matmul
                        # instead of a full log-tree on the DVE queue
                        ginc = (not NOL) and (
                            (h == NREP - 1 and qc % 2 == 1)
                            or (qc == 3 and h == NREP - 2)
                        )
                        if ginc:
                            if qc not in ft_reg:
                                ft_reg[qc] = consts.tile(
                                    [128, 8, 512], BF16, tag="ftree",
                                    name=f"ft{qc}",
                                )
                        for kt in range(nkt):
                            d = kt - 4 * qc
                            # diagonal blocks: columns q < 128d are fully
                            # masked (k+128d<=q never holds) — skip them in
                            # score/exp; zero them for the l tree
                            lo = 128 * d if d > 0 else 0
                            sT_ps = psum.tile([128, 512], F32, tag="bank")
                            nc.tensor.matmul(
                                sT_ps[:, lo:512],
                                lhsT=kT_t[kt // 4][:, 128 * (kt % 4):128 * (kt % 4 + 1)],
                                rhs=qT_t[qc][:, h, lo:512],
                                start=True,
                                stop=True,
                            )
                            nc.scalar.activation(
                                et[:, b + kt, lo:512], sT_ps[:, lo:512],
                                mybir.ActivationFunctionType.Exp,
                                scale=SCALE,
                            )
                            if d >= 0:
                                # mask this diagonal tile immediately: deps
                                # are byte-range precise, so its y-matmul
                                # waits only on this tile's exp+mask.
                                # Alternate DVE/Pool so neither queue
                                # saturates against its other work.
                                if lo > 0:
                                    # prefix must be finite zero for the l
                                    # tree: first-touch SBUF can hold NaNs
                                    nc.vector.memset(et[:, b + kt, 0:lo], 0.0)
                                nc.vector.tensor_mul(
                                    et[:, b + kt, lo:512],
                                    et[:, b + kt, lo:512],
                                    mask_sb[:, d, lo:512],
                                )
                            if ginc:
                                f = ft_reg[qc]
                                fb = 0 if h == NREP - 1 else 4
                                if kt < 4:
                                    nc.vector.tensor_copy(
                                        f[:, fb + kt, :], et[:, b + kt, :]
                                    )
                                else:
                                    nc.vector.tensor_add(
                                        f[:, fb + kt % 4, :],
                                        f[:, fb + kt % 4, :],
                                        et[:, b + kt, :],
                                    )

                def att_B(qc, heads):
                    nkt = 4 * qc + 4
                    half, dst0 = qc // 2, 4 * (qc % 2)
                    yv = y_in[half].rearrange("(d r) m -> r d m", r=BLK)
                    if qc not in l4_reg:
                        # single-partition tile: head h -> cols [512h,512h+512)
                        # (flat (h,d,m) order matches the strided l DMA dest);
                        # ACT can't write at partition starts other than 0/64
                        l4_reg[qc] = outp.tile(
                            [1, 4, 512], BF16, tag="l4", name=f"l4_{qc}"
                        )
                    l4 = l4_reg[qc]
                    for h in heads:
                        et, b = att_etile(qc, h)
                        yT_ps = psum.tile([128, 512], F32, tag="bank")
                        for kt in range(nkt - 4):
                            nc.tensor.matmul(
                                yT_ps,
                                lhsT=vkd_t[kt // 4][:, kt % 4, :],
                                rhs=et[:, b + kt, :],
                                start=(kt == 0),
                                stop=False,
                            )
                        for kt in range(nkt - 4, nkt):
                            d = kt - 4 * qc
                            lo = 128 * d if d > 0 else 0
                            nc.tensor.matmul(
                                yT_ps[:, lo:512],
                                lhsT=vkd_t[kt // 4][:, kt % 4, :],
                                rhs=et[:, b + kt, lo:512],
                                start=(kt == 0),
                                stop=(kt == nkt - 1),
                            )
                        # y ships UNNORMALIZED; the softmax denominator is
                        # partition-summed on the idle GpSimd (off the PE/DVE
                        # critical path) and rides along in the A2A payload
                        yn_sb = outp.tile([128, 512], BF16, tag="yn")
                        if qc >= 2:
                            nc.vector.tensor_copy(yn_sb, yT_ps)
                        else:
                            nc.scalar.copy(yn_sb, yT_ps)
                        if not NOL:
                            if (h == NREP - 1 and qc % 2 == 1) or (
                                qc == 3 and h == NREP - 2
                            ):
                                f = ft_reg[qc]
                                fb = 0 if h == NREP - 1 else 4
                                nc.vector.tensor_add(
                                    f[:, fb:fb + 2, :], f[:, fb:fb + 2, :],
                                    f[:, fb + 2:fb + 4, :],
                                )
                                nc.vector.tensor_add(
                                    f[:, fb:fb + 1, :], f[:, fb:fb + 1, :],
                                    f[:, fb + 1:fb + 2, :],
                                )
                                lsrc, lb = f, fb
                            else:
                                # in-place log-tree sum over the nkt tiles
                                # (bf16, 2x DVE rate; after y matmuls read e)
                                w = nkt
                                while w > 1:
                                    h2 = w // 2
                                    nc.vector.tensor_add(
                                        et[:, b:b + h2, :], et[:, b:b + h2, :],
                                        et[:, b + h2:b + 2 * h2, :],
                                    )
                                    if w % 2 == 1:
                                        nc.vector.tensor_add(
                                            et[:, b:b + 1, :], et[:, b:b + 1, :],
                                            et[:, b + w - 1:b + w, :],
                                        )
                                    w = h2
                                lsrc, lb = et, b
                            l_acc = work.tile([128, 512], F32, tag="lacc")
                            nc.vector.tensor_copy(l_acc, lsrc[:, lb, :])
                            l_red = work.tile([128, 512], F32, tag="lred")
                            nc.gpsimd.partition_all_reduce(
                                l_red, l_acc, channels=128,
                                reduce_op=bass_isa.ReduceOp.add,
                            )
                            nc.scalar.copy(l4[0:1, h, :], l_red[0:1, :])
                        # scatter to the A2A buffer: the 4 dest blocks are
                        # at regular BLK strides -> ONE strided DMA
                        nc.sync.dma_start(
                            out=yv[128 * h:128 * (h + 1), dst0:dst0 + 4, :],
                            in_=yn_sb,
                        )
                        if not NOL and h == NREP - 1:
                            # all 4 heads' denominators in one strided DMA
                            nc.sync.dma_start(
                                out=yv[512:516, dst0:dst0 + 4, :], in_=l4
                            )

                def attention_chunk(qc):
                    for h in range(NREP):
                        att_A(qc, [h])
                        att_B(qc, [h])

                def a2a(half):
                    if LOCALA2A:
                        nc.sync.dma_start(out=y_out[half][:, :], in_=y_in[half][:, :])
                        return
                    nc.gpsimd.collective_compute(
                        "AllToAll",
                        mybir.AluOpType.bypass,
                        ins=[y_in[half][:, :]],
                        outs=[y_out[half][:, :]],
                        replica_groups=[list(range(N_CORES))],
                    )

                def load_norm(half):
                    # y load + softmax normalization for one out-row half;
                    # runs right after its A2A, overlapping later work.
                    # 8 block DMAs split across the ACT/SP issue queues.
                    yh = weights.tile(
                        [128, NKT, 128], BF16, tag=f"ysb{half}", name=f"yh{half}"
                    )
                    l_sb = consts.tile(
                        [1, NKT * 128], BF16, tag="lsb", name=f"lsb{half}"
                    )
                    yv = y_out[half].rearrange("(d r) m -> d r m", r=BLK)
                    for d in range(8):
                        eng = nc.scalar if (half == 1 and d % 2 == 1) else nc.sync
                        eng.dma_start(
                            out=yh[:, 4 * d:4 * (d + 1), :],
                            in_=yv[d, 0:512, :].rearrange("(n p) m -> p n m", p=128),
                        )
                    # all 32 head-rows of l in one strided DMA, flat order
                    # (block, head, col) == l_sb column order 512c+128h+m
                    eng = nc.scalar if half == 1 else nc.sync
                    eng.dma_start(out=l_sb, in_=yv[:, 512:516, :])
                    # normalize: column q of kt-tile kt scales by 1/l[kt][q];
                    # flat offset of tile kt's l is 128*kt
                    for kt in range(NKT):
                        lr = work.tile([1, 128], F32, tag="lr")
                        nc.vector.reciprocal(
                            lr, l_sb[0:1, 128 * kt:128 * (kt + 1)]
                        )
                        rb = work.tile([128, 128], F32, tag="rb")
                        nc.gpsimd.partition_broadcast(rb, lr[0:1, :])
                        nc.vector.tensor_mul(
                            yh[:, kt, :], yh[:, kt, :], rb
                        )
                    return yh

                # interleave: attention chunk qc only needs phase-1
                # chunks 0..qc, and placing it early in program order lets
                # it claim PSUM banks / DVE / ACT while later phase-1
                # chunks keep the PE busy
                allh = list(range(NREP))
                p1_chunk(0)
                p1_chunk(1)
                if MAXPHASE >= 2:
                    att_A(0, allh)
                p1_chunk(2)
                if MAXPHASE >= 2:
                    att_B(0, allh)
                    att_A(1, allh)
                p1_chunk(3)

                # ---------- phase 4 machinery (emission interleaves with
                # the attention tail below) ----------
                # wo chunk = [512 rows x 1024 cols] as [128, 8, 512]
                # (dim1 = 4c+kn), loaded by two 0.5MB DMAs split across the
                # SP/ACT issue queues. Chunks 0-5 of group 0 are RESIDENT in
                # retired rings (x0,x1,wq0-3 — disjoint lifetimes); all
                # other chunks stream through dead e-tile halves (4-deep).
                yh_sb = {}
                g0_slots = []
                wo_stream = {"tile": None, "n": 0}

                def wo_slot_stream():
                    if wo_stream["n"] % 2 == 0:
                        wo_stream["tile"] = expp.tile(
                            [128, 16, 512], BF16, tag="efull",
                            name=f"wos{wo_stream['n']}",
                        )
                    t = wo_stream["tile"]
                    ofs = 8 * (wo_stream["n"] % 2)
                    wo_stream["n"] += 1
                    return t, ofs

                def load_wo(g, kt4, tile_, ofs, eng2=None):
                    # one 1MB DMA with 2KB-contiguous rows (descriptor
                    # granularity dominates HW DMA efficiency; the [4,1024]
                    # source view lands flat on the tile's [8,512] cols)
                    rs = slice(512 * kt4, 512 * (kt4 + 1))
                    eng = eng2 or nc.scalar
                    eng.dma_start(
                        out=tile_[:, ofs:ofs + 8, :],
                        in_=woT[rs, 1024 * g:1024 * (g + 1)].rearrange(
                            "(n p) m -> p n m", p=128),
                    )

                def p4_mms(g, kt4, tile_, ofs, half, o_ps):
                    for kn in range(4):
                        kt = 4 * kt4 + kn
                        for c in range(2):
                            nc.tensor.matmul(
                                o_ps[(half, c)],
                                lhsT=yh_sb[half][:, kt, :],
                                rhs=tile_[:, ofs + 2 * kn + c, :],
                                start=(kt == 0),
                                stop=(kt == NKT - 1),
                            )

                def p4_out(g, halves, o_ps, fine=False, act_only=False):
                    pieces = (0, 256) if fine else (0,)
                    w = 256 if fine else 512
                    for i, half in enumerate(halves):
                        for c in range(2):
                            o_sb = outp.tile([128, 512], F32, tag="osb")
                            for j, p0 in enumerate(pieces):
                                if act_only or (i + c + j) % 2 == 0:
                                    nc.scalar.copy(
                                        o_sb[:, p0:p0 + w],
                                        o_ps[(half, c)][:, p0:p0 + w],
                                    )
                                else:
                                    nc.vector.tensor_copy(
                                        o_sb[:, p0:p0 + w],
                                        o_ps[(half, c)][:, p0:p0 + w],
                                    )
                                eng = nc.sync if (c + j) % 2 == 0 else nc.scalar
                                eng.dma_start(
                                    out=out[
                                        128 * half:128 * (half + 1),
                                        1024 * g + 512 * c + p0:
                                        1024 * g + 512 * c + p0 + w,
                                    ],
                                    in_=o_sb[:, p0:p0 + w],
                                )

                def o_alloc(g, halves):
                    return {
                        (half, c): psum.tile(
                            [128, 512], F32, tag="bank",
                            name=f"o{g}_{half}_{c}",
                        )
                        for half in halves for c in range(2)
                    }

                def g0_chunk(k, o_ps):
                    if k < 2:
                        t_ = stream.tile([128, 8, 512], BF16, tag=f"x{k}",
                                         name=f"wog0_{k}")
                        g0_slots.append((t_, 0))
                    elif k < 6:
                        t_ = weights.tile([128, 8, 512], BF16, tag=f"wq{k - 2}",
                                          name=f"wog0_{k}")
                        g0_slots.append((t_, 0))
                    else:
                        t_, ofs = wo_slot_stream()
                        load_wo(0, k, t_, ofs, eng2=nc.sync)
                        p4_mms(0, k, t_, ofs, 0, o_ps)
                        return
                    load_wo(0, k, t_, 0, eng2=nc.sync)
                    p4_mms(0, k, t_, 0, 0, o_ps)

                # ---------- attention tail: chunks 2/3 head-pipelined (the
                # next head's score matmuls fill the ACT-bound exp lag of
                # the previous head's y matmuls); group-0 half-0 wo chunks
                # interleave into chunk 3 as extra PE filler ----------
                if MAXPHASE >= 2:
                    att_B(1, allh)
                    if MAXPHASE >= 3:
                        a2a(0)
                    att_A(2, [0])
                    att_A(2, [1])
                    att_B(2, [0])
                    att_A(2, [2])
                    att_B(2, [1])
                    att_A(2, [3])
                    att_B(2, [2])
                    att_B(2, [3])
                    if MAXPHASE >= 4:
                        yh_sb[0] = load_norm(0)
                        o0a = o_alloc(0, [0])
                        att_A(3, [0])
                        g0_chunk(0, o0a)
                        att_A(3, [1])
                        att_B(3, [0])
                        g0_chunk(1, o0a)
                        att_A(3, [2])
                        att_B(3, [1])
                        g0_chunk(2, o0a)
                        att_A(3, [3])
                        att_B(3, [2])
                        g0_chunk(3, o0a)
                        att_B(3, [3])
                        if MAXPHASE >= 3:
                            a2a(1)
                        yh_sb[1] = load_norm(1)
                        # runway: remaining half-0-only chunks the in-order
                        # PE queue chews while A2A#2 -> normalize completes
                        for k in (4, 5, 6, 7):
                            g0_chunk(k, o0a)
                        p4_out(0, [0], o0a, act_only=True)
                        # groups 1-3, both halves, streaming wo
                        for g in (1, 2, 3):
                            og = o_alloc(g, [0, 1])
                            for kt4 in range(8):
                                t_, ofs = wo_slot_stream()
                                load_wo(g, kt4, t_, ofs)
                                for half in range(2):
                                    p4_mms(g, kt4, t_, ofs, half, og)
                            p4_out(g, [0, 1], og)
                        # group 0, half 1: resident chunks 0-5 need no DMA;
                        # 6-7 restream (2MB)
                        o0b = o_alloc(0, [1])
                        for k in range(6):
                            t_, ofs = g0_slots[k]
                            p4_mms(0, k, t_, ofs, 1, o0b)
                        for k in (6, 7):
                            t_, ofs = wo_slot_stream()
                            load_wo(0, k, t_, ofs)
                            p4_mms(0, k, t_, ofs, 1, o0b)
                        p4_out(0, [1], o0b, fine=True)
                    else:
                        # att(2) was already emitted pipelined above
                        attention_chunk(3)
                        if MAXPHASE >= 3:
                            a2a(1)

    nc.compile()
    _cached["nc"] = nc
    return nc


def _build_in_maps(inputs):
    return _shard_inputs(**inputs)


def _shard_inputs(x, cos, sin, wq, wk, wv, wo, start_pos):
    x = np.asarray(x, dtype=np.float32)
    cos = np.asarray(cos, dtype=np.float32)
    sin = np.asarray(sin, dtype=np.float32)
    wq = np.asarray(wq, dtype=np.float32)
    wk = np.asarray(wk, dtype=np.float32)
    wv = np.asarray(wv, dtype=np.float32)
    wo = np.asarray(wo, dtype=np.float32)
    sp = int(start_pos)

    xT = np.ascontiguousarray(x[0].T).astype(NPBF16)  # (DIM, T)
    cosT = np.ascontiguousarray(cos[sp:sp + T].T)  # (64, T)
    sinT = np.ascontiguousarray(sin[sp:sp + T].T)
    cos2 = np.concatenate([cosT, cosT], axis=0)  # (128, T)
    sin2 = np.concatenate([-sinT, sinT], axis=0)  # rotate-half signs folded in

    kk = np.arange(128)[:, None]
    qq = np.arange(512)[None, :]
    masks = np.concatenate(
        [(kk + 128 * d <= qq).astype(NPBF16) for d in range(4)], axis=1
    )  # (128, 2048)

    woT = np.ascontiguousarray(wo.T).astype(NPBF16)  # (DIM, DIM), full

    in_maps = []
    for c in range(N_CORES):
        qrows = slice(NREP * HD * c, NREP * HD * (c + 1))
        krows = slice(HD * c, HD * (c + 1))
        in_maps.append({
            "xT": xT,
            "cos2": cos2,
            "sin2": sin2,
            "masks": masks,
            "wqT": np.ascontiguousarray(wq[qrows, :].T).astype(NPBF16),
            "wkvT": np.ascontiguousarray(
                np.concatenate([wk[krows, :], wv[krows, :]], axis=0).T
            ).astype(NPBF16),
            "woT": woT,
        })
    return in_maps


def kernel(x, cos, sin, wq, wk, wv, wo, start_pos):
    in_maps = _shard_inputs(x, cos, sin, wq, wk, wv, wo, start_pos)
    nc = _build_kernel()
    res = run_bass_kernel_spmd(nc, in_maps, core_ids=list(range(N_CORES)))
    # core d returns T rows [128d,128d+128) and [1024+128d,1024+128d+128)
    full = np.empty((T, DIM), np.float32)
    for d in range(N_CORES):
        o = res.results[d]["out"]
        full[128 * d:128 * (d + 1)] = o[0:128]
        full[1024 + 128 * d:1024 + 128 * (d + 1)] = o[128:256]
    return full.reshape(1, T, DIM)
